# revision 1
# baseline (speedup 1.0000x reference)
"""Trainium2 Bass kernel for ConstOutputFilteredNormalized (segment_reduce).

y[i, j] = (x[i, j] != 0 ? f[j] : 0) / rowsum_j(masked_f[i, :]), with rows whose
masked sum is exactly 0 producing exactly 0.

Strategy: data-parallel over the batch axis — 16384 rows split into 8 shards of
2048 rows, one per NeuronCore; f (4096 floats) replicated to all cores and
broadcast across the 128 SBUF partitions once. Each core processes 16 tiles of
[128 rows, 4096 cols]:
  DMA load x tile -> DVE scalar_tensor_tensor: y = (x != 0) * f with fused
  row-sum accumulator -> safe-denominator fixup + reciprocal on [128,1] ->
  ScalarE per-row scale y *= 1/denom -> DMA store y tile.
The kernel is HBM-bandwidth-bound (64 MiB of traffic per core).
"""

import numpy as np

B, N = 16384, 4096
NCORES = 8
ROWS_PER_CORE = B // NCORES  # 2048
P = 128
NTILES = ROWS_PER_CORE // P  # 16

_cache = {}


def _build():
    import concourse.bass as bass
    import concourse.tile as tile
    from concourse import bacc, mybir

    nc = bacc.Bacc(
        "TRN2",
        target_bir_lowering=False,
        debug=False,
        num_devices=NCORES,
    )
    f32 = mybir.dt.float32
    x_d = nc.dram_tensor("x", [ROWS_PER_CORE, N], f32, kind="ExternalInput").ap()
    f_d = nc.dram_tensor("f", [N], f32, kind="ExternalInput").ap()
    y_d = nc.dram_tensor("y", [ROWS_PER_CORE, N], f32, kind="ExternalOutput").ap()

    with tile.TileContext(nc) as tc:
        with (
            tc.tile_pool(name="consts", bufs=1) as consts,
            tc.tile_pool(name="xp", bufs=4) as xp,
            tc.tile_pool(name="yp", bufs=4) as yp,
            tc.tile_pool(name="sp", bufs=8) as sp,
        ):
            # Replicate f across all 128 partitions with a stride-0 DMA.
            f_sb = consts.tile([P, N], f32)
            f_bcast = bass.AP(
                tensor=f_d.tensor,
                offset=f_d.offset,
                ap=[[0, P], f_d.ap[0]],
            )
            nc.gpsimd.dma_start(out=f_sb[:], in_=f_bcast)

            for i in range(NTILES):
                rows = slice(i * P, (i + 1) * P)
                x_t = xp.tile([P, N], f32)
                nc.sync.dma_start(out=x_t[:], in_=x_d[rows, :])

                y_t = yp.tile([P, N], f32)
                den = sp.tile([P, 1], f32)
                # y = (x != 0) * f ; den = rowsum(y)
                nc.vector.scalar_tensor_tensor(
                    out=y_t[:],
                    in0=x_t[:],
                    scalar=0.0,
                    in1=f_sb[:],
                    op0=mybir.AluOpType.not_equal,
                    op1=mybir.AluOpType.mult,
                    accum_out=den[:],
                )
                # safe = den + (den == 0); recip = 1 / safe
                safe = sp.tile([P, 1], f32)
                nc.vector.tensor_scalar(
                    out=safe[:],
                    in0=den[:],
                    scalar1=0.0,
                    scalar2=None,
                    op0=mybir.AluOpType.is_equal,
                )
                nc.vector.tensor_add(out=safe[:], in0=safe[:], in1=den[:])
                nc.vector.reciprocal(out=safe[:], in_=safe[:])
                # y *= recip (per-partition scalar broadcast) on ScalarE
                nc.scalar.mul(y_t[:], y_t[:], safe[:])

                nc.sync.dma_start(out=y_d[rows, :], in_=y_t[:])

    nc.compile()
    return nc


def kernel(x: np.ndarray, f: np.ndarray) -> np.ndarray:
    from concourse.bass_utils import run_bass_kernel_spmd

    if "nc" not in _cache:
        _cache["nc"] = _build()
    nc = _cache["nc"]

    x = np.ascontiguousarray(x, dtype=np.float32)
    f = np.ascontiguousarray(f, dtype=np.float32)
    assert x.shape == (B, N) and f.shape == (N,)

    shards = np.split(x, NCORES, axis=0)
    in_maps = [{"x": s, "f": f} for s in shards]
    res = run_bass_kernel_spmd(nc, in_maps, list(range(NCORES)))
    return np.concatenate([res.results[i]["y"] for i in range(NCORES)], axis=0)


# revision 15
# speedup vs baseline: 47299.8886x; 47299.8886x over previous
"""Trainium2 Bass kernel for ConstOutputFilteredNormalized (segment_reduce).

y[i, j] = (x[i, j] != 0 ? f[j] : 0) / rowsum_j(masked_f[i, :]), with rows whose
masked sum is exactly 0 producing exactly 0.

Strategy: data-parallel over the batch axis — 16384 rows split into 8 shards of
2048 rows, one per NeuronCore; f (4096 floats) replicated to all cores and
broadcast across the 128 SBUF partitions once. Each core processes 16 tiles of
[128 rows, 4096 cols]. The kernel is HBM-bandwidth-bound (64 MiB per core).

Denominator precision: a plain fp32 running sum of masked f loses ~1e-5
absolute, which is catastrophic for rows whose masked sum nearly cancels
(min |denom| in the reference data is ~5e-3). We split f = f_hi + f_lo where
f_hi = f rounded to the 2^-10 grid (via the (f + 12288) - 12288 trick) and
f_lo = f - f_hi; both splits are exact in fp32. Then:
  - sum(mask * f_hi) is EXACT in fp32 in any order (all terms are multiples
    of 2^-10 and the running sum stays far below 2^24 * 2^-10),
  - sum(mask * f_lo) carries only ~1e-9 error (terms are < 2^-11),
so den = den_hi + den_lo is accurate to ~1 ulp, and y = y_hi + y_lo
reconstructs mask * f exactly (f_hi + f_lo == f elementwise in fp32).

Engine assignment per tile (only op/engine pairs validated on real walrus:
GpSimd supports TensorTensor but NOT TensorScalar/ScalarTensorTensor):
  DVE  STT1: y = (x != 0) * f_hi, accum den_hi      (exact grid sum)
  DVE  STT2: x <- (x != 0) * f_lo in-place, accum den_lo
  DVE  small: den = hi + lo; safe = den + (den == 0); recip = 1/safe
  Pool TT:   y[:, :3072] += x[:, :3072]   (y becomes mask*f exactly)
  DVE  TT:   y[:, 3072:] += x[:, 3072:]
  ACT  y *= recip (per-partition broadcast)
DVE ~10us, Pool ~6.2us, ACT ~3.6us vs DMA ~11.7us per tile -> DMA-bound.

Scheduling: software-pipelined emission (lag 1) so each tile's reduction tail
isn't buried behind the next tiles' bulk ops in the DVE program order; loads
issue from the SP sequencer and stores from the ACT sequencer (two
independent HWDGE rings) so a store waiting on its data never blocks loads.
"""

import numpy as np

B, N = 16384, 4096
NCORES = 8
ROWS_PER_CORE = B // NCORES  # 2048
P = 128
SPLIT_C = 12288.0  # 1.5 * 2^13: rounds fp32 in (-4096, 4096) to the 2^-10 grid
GP_COLS = 3072  # columns of the reconstruction add done on GpSimd (rest: DVE)

_cache = {}


def _build(rows_per_core=ROWS_PER_CORE):
    import concourse.bass as bass
    import concourse.tile as tile
    from concourse import bacc, mybir

    ntiles = rows_per_core // P
    nc = bacc.Bacc(
        "TRN2",
        target_bir_lowering=False,
        debug=False,
        num_devices=NCORES,
    )
    f32 = mybir.dt.float32
    x_d = nc.dram_tensor("x", [rows_per_core, N], f32, kind="ExternalInput").ap()
    f_d = nc.dram_tensor("f", [N], f32, kind="ExternalInput").ap()
    y_d = nc.dram_tensor("y", [rows_per_core, N], f32, kind="ExternalOutput").ap()

    with tile.TileContext(nc) as tc:
        with (
            tc.tile_pool(name="consts", bufs=1) as consts,
            tc.tile_pool(name="xp", bufs=5) as xp,
            tc.tile_pool(name="yp", bufs=4) as yp,
            tc.tile_pool(name="sp", bufs=4) as sp,
        ):
            # Replicate f across all 128 partitions with a stride-0 DMA,
            # into a transient tile borrowed from the y pool.
            f_sb = yp.tile([P, N], f32, tag="y_t")
            f_bcast = bass.AP(
                tensor=f_d.tensor,
                offset=f_d.offset,
                ap=[[0, P], f_d.ap[0]],
            )
            nc.gpsimd.dma_start(out=f_sb[:], in_=f_bcast)

            # f_hi = round_to_2^-10_grid(f) = (f + C) - C; f_lo = f - f_hi
            # (both exact).
            f_hi = consts.tile([P, N], f32)
            nc.vector.tensor_scalar(
                out=f_hi[:],
                in0=f_sb[:],
                scalar1=SPLIT_C,
                scalar2=SPLIT_C,
                op0=mybir.AluOpType.add,
                op1=mybir.AluOpType.subtract,
            )
            f_lo = consts.tile([P, N], f32)
            nc.vector.tensor_sub(out=f_lo[:], in0=f_sb[:], in1=f_hi[:])

            live = []  # tiles awaiting their back half

            def back_half(x_t, y_t, dhi, dlo, rows):
                # den = dhi + dlo; safe = den + (den == 0); recip = 1/safe
                den = sp.tile([P, 1], f32)
                nc.vector.tensor_add(out=den[:], in0=dhi[:], in1=dlo[:])
                safe = sp.tile([P, 1], f32)
                nc.vector.tensor_scalar(
                    out=safe[:],
                    in0=den[:],
                    scalar1=0.0,
                    scalar2=None,
                    op0=mybir.AluOpType.is_equal,
                )
                nc.vector.tensor_add(out=safe[:], in0=safe[:], in1=den[:])
                nc.vector.reciprocal(out=safe[:], in_=safe[:])
                # y += y_lo: reconstruct mask*f exactly (split Pool / DVE)
                nc.gpsimd.tensor_add(
                    out=y_t[:, :GP_COLS],
                    in0=y_t[:, :GP_COLS],
                    in1=x_t[:, :GP_COLS],
                )
                nc.vector.tensor_add(
                    out=y_t[:, GP_COLS:],
                    in0=y_t[:, GP_COLS:],
                    in1=x_t[:, GP_COLS:],
                )
                # y *= recip (per-partition scalar broadcast) on ScalarE
                nc.scalar.mul(y_t[:], y_t[:], safe[:])
                nc.scalar.dma_start(out=y_d[rows, :], in_=y_t[:])

            for i in range(ntiles):
                rows = slice(i * P, (i + 1) * P)
                x_t = xp.tile([P, N], f32)
                nc.sync.dma_start(out=x_t[:], in_=x_d[rows, :])

                y_t = yp.tile([P, N], f32)
                dhi = sp.tile([P, 1], f32)
                # y = (x != 0) * f_hi ; dhi = rowsum (exact in any order)
                nc.vector.scalar_tensor_tensor(
                    out=y_t[:],
                    in0=x_t[:],
                    scalar=0.0,
                    in1=f_hi[:],
                    op0=mybir.AluOpType.not_equal,
                    op1=mybir.AluOpType.mult,
                    accum_out=dhi[:],
                )
                # x <- (x != 0) * f_lo in-place ; dlo = rowsum (tiny terms)
                dlo = sp.tile([P, 1], f32)
                nc.vector.scalar_tensor_tensor(
                    out=x_t[:],
                    in0=x_t[:],
                    scalar=0.0,
                    in1=f_lo[:],
                    op0=mybir.AluOpType.not_equal,
                    op1=mybir.AluOpType.mult,
                    accum_out=dlo[:],
                )
                live.append((x_t, y_t, dhi, dlo, rows))
                if len(live) > 1:
                    back_half(*live.pop(0))
            while live:
                back_half(*live.pop(0))

    nc.compile()
    return nc


def kernel(x: np.ndarray, f: np.ndarray) -> np.ndarray:
    from concourse.bass_utils import run_bass_kernel_spmd

    if "nc" not in _cache:
        _cache["nc"] = _build()
    nc = _cache["nc"]

    x = np.ascontiguousarray(x, dtype=np.float32)
    f = np.ascontiguousarray(f, dtype=np.float32)
    assert x.shape == (B, N) and f.shape == (N,)

    shards = np.split(x, NCORES, axis=0)
    in_maps = [{"x": s, "f": f} for s in shards]
    res = run_bass_kernel_spmd(nc, in_maps, list(range(NCORES)))
    return np.concatenate([res.results[i]["y"] for i in range(NCORES)], axis=0)
